# revision 1
# baseline (speedup 1.0000x reference)
"""GATv2FeatureExtractor Trainium2 kernel (8 NeuronCores, edge-parallel by dst).

Strategy
--------
Edges are sorted by destination and sharded into 8 contiguous dst ranges (6250
nodes per core).  Within each core the local node order is a degree-balanced
permutation so every fixed node-window (55 nodes for GAT layer 1, 119 for
layer 2) carries a near-equal edge count; the weighted message scatter is a
one-hot matmul accumulating into a PSUM window, so no cross-core reduction is
needed.  One collective runs: an AllGather of the layer-2 source table
(h1 @ Wl2) between the layers.  Gather indices are remapped on the host so the
SPMD program is identical on all cores.

GATv2 math: no per-edge softmax max pass is needed (alpha is O(1) for this
model; exp() cannot overflow given the fixed -4 bias).  The attention logit
uses the scalar engine's parametric_relu (AF.Prelu, alpha=0.2) directly on
the per-edge message matmul output, then per-head dot-accumulate against a
tiled att row on the vector engine.  All biases are folded into matmul
ones-rows.  The AllGather of the layer-2 source table is split into 5
window-aligned chunks fired during layer 1 so it overlaps compute.

Compute runs in 16-bit (fp32 PSUM accumulation).  Per-chunk s/payload
matmuls share one [128, 512] f32 PSUM bank (tag ring depth 4) so TensorE
runs several chunks ahead of the scalar/vector consumers; edge windows use
variable per-window chunk counts to avoid padding to the global max.
"""

import os
import sys

import numpy as np

if os.path.isdir("/opt/trn_rl_repo") and "/opt/trn_rl_repo" not in sys.path:
    sys.path.insert(0, "/opt/trn_rl_repo")

import concourse.bacc as bacc
import concourse.bass as bass
import concourse.mybir as mybir
import concourse.tile as tile
from concourse.bass import IndirectOffsetOnAxis
from concourse.bass_utils import run_bass_kernel_spmd

F32 = mybir.dt.float32
BF16 = mybir.dt.float16  # 16-bit compute dtype (fp16: better mantissa, same speed)
I32 = mybir.dt.int32
AF = mybir.ActivationFunctionType
ALU = mybir.AluOpType

NCORES = 8
P = 128

F_IN, ED, HID, H, OUT = 32, 8, 64, 4, 256
HC = H * HID  # 256
XW = HC + 2   # xl2 table width: 256 payload + t_lin hi/lo

NPF = np.float32
NPB = "bfloat16"  # via ml_dtypes below

def _bf(a):
    """Cast numpy f32 array to the 16-bit compute dtype."""
    return np.asarray(a, np.float32).astype(np.float16)


def _ceil_div(a, b):
    return -(-a // b)


# ----------------------------------------------------------------------------
# host-side preprocessing
# ----------------------------------------------------------------------------

def _balanced_perms(deg, n, npc, span):
    """Per-core degree-balanced local permutation for `span`-node windows."""
    perms, invs = [], []
    for c in range(NCORES):
        d = deg[c * npc:(c + 1) * npc]
        order = np.argsort(-d, kind="stable")
        rows = np.arange(npc)
        seq = np.lexsort((rows // span, rows % span))
        perm = np.empty(npc, np.int64)
        perm[seq] = order
        inv = np.empty(npc, np.int64)
        inv[perm] = np.arange(npc)
        perms.append(perm)
        invs.append(inv)
    return perms, invs


def _gather_meta(all_rows, all_winid, nw, half):
    """Per-window lo/hi chunk counts (max over cores); rows < half gather
    from a table prefix whose producers finish early, enabling overlap."""
    klo = np.zeros(nw, np.int64)
    khi = np.zeros(nw, np.int64)
    for rows, winid in zip(all_rows, all_winid):
        lo_ct = np.bincount(winid[rows < half], minlength=nw)
        hi_ct = np.bincount(winid[rows >= half], minlength=nw)
        klo = np.maximum(klo, _ceil_div(lo_ct, P))
        khi = np.maximum(khi, _ceil_div(hi_ct, P))
    co = np.zeros(nw + 1, np.int64)
    co[1:] = np.cumsum(klo + khi)
    return klo.tolist(), khi.tolist(), co.tolist()


def _pack_layer(rows, dstl, winid, ea_e, nw, klo, khi, co, half):
    """Pack one core's edges into variable-k chunk-major arrays + int16 idxs.

    Slot order per window: lo-half edges (idx = row), zero-pad to klo*P,
    then hi-half edges (idx = row-HALF), zero-pad to end.  Pad slots gather
    table row 0 (valid data) and carry dstl = -1 so the scatter ignores them.
    """
    tc = co[-1]
    idx_f = np.zeros(tc * P, np.int32)
    dst_f = np.full(tc * P, -1.0, np.float32)
    ea_f = np.zeros((tc * P, ED), np.float32)
    order = np.argsort(winid, kind="stable")
    bounds = np.searchsorted(winid[order], np.arange(nw + 1))
    for w in range(nw):
        a, b = bounds[w], bounds[w + 1]
        if a == b:
            continue
        sel = order[a:b]
        r = rows[sel]
        hi = r >= half
        base = co[w] * P
        pos_lo = base + np.arange((~hi).sum())
        pos_hi = base + klo[w] * P + np.arange(hi.sum())
        pos = np.concatenate([pos_lo, pos_hi])
        sel = np.concatenate([sel[~hi], sel[hi]])
        idx_f[pos] = rows[sel].astype(np.int32)
        dst_f[pos] = dstl[sel]
        ea_f[pos] = ea_e[sel]
    idx_a = idx_f.reshape(tc, P).T.copy()  # [P, tc] int32 rows for indirect DMA
    dst_a = dst_f.reshape(tc, P).T.copy()
    ea_a = _bf(ea_f.reshape(tc, P, ED).transpose(1, 0, 2).reshape(P, tc * ED))
    return idx_a, dst_a, ea_a


def _prep_host(inputs, n, e, npc, w1, w2):
    x = np.asarray(inputs["x"], np.float32)
    ei = np.asarray(inputs["edge_index"])
    ea = np.asarray(inputs["edge_attr"], np.float32)
    src = ei[0].astype(np.int64)
    dst = ei[1].astype(np.int64)

    deg = np.bincount(dst, minlength=n)
    perms, invs = _balanced_perms(deg, n, npc, w1)
    nw1 = _ceil_div(npc, w1)
    nw2 = _ceil_div(npc, w2)

    owner = dst // npc
    # per-core edge sets
    core_edges = [np.where(owner == c)[0] for c in range(NCORES)]
    core_pack = []
    for c in range(NCORES):
        es = core_edges[c]
        r = invs[c][dst[es] - c * npc]      # balanced local row of dst
        core_pack.append((es, r, r // w1, r // w2))

    # AllGather chunk boundaries (rows of xl2loc), aligned to w1 windows so
    # each chunk's collective can fire as soon as its windows are written.
    NAG = 5
    agw = sorted({round(nw1 * j / NAG) for j in range(NAG + 1)})
    agb = [min(w1 * wj, npc) for wj in agw]

    inv_all = np.concatenate(invs)  # inv_all[o*npc + local_id] = local row in core o
    rows1_all, rows2_all = [], []
    for c in range(NCORES):
        es, r, w1id, w2id = core_pack[c]
        lo = c * npc
        g = src[es]
        v = (g - lo) % n
        row1 = np.where(v < npc, invs[c][np.minimum(v, npc - 1)], v)  # L1 table row
        # xl2full row under the chunk-major AllGather layout
        l2 = inv_all[g]
        o2 = g // npc
        j2 = np.searchsorted(agb, l2, side="right") - 1
        lens = np.asarray(agb[1:] + [npc]) - np.asarray(agb)
        row2 = NCORES * np.asarray(agb)[j2] + o2 * lens[j2] + (l2 - np.asarray(agb)[j2])
        rows1_all.append(row1.astype(np.int64))
        rows2_all.append(row2.astype(np.int64))

    # L2 early/late split: sources in the first NAG-1 AllGather chunks
    # (rows < b2s of the chunk-major table) can be gathered before the
    # final collective lands.
    b2s = NCORES * agb[-2]
    klo1, khi1, co1 = _gather_meta(rows1_all, [cp[2] for cp in core_pack], nw1,
                                   1 << 31)
    klo2, khi2, co2 = _gather_meta(rows2_all, [cp[3] for cp in core_pack], nw2,
                                   1 << 31)
    packed1, packed2 = [], []
    for c in range(NCORES):
        es, r, w1id, w2id = core_pack[c]
        packed1.append(_pack_layer(rows1_all[c], (r - w1id * w1), w1id, ea[es],
                                   nw1, klo1, khi1, co1, 1 << 31))
        packed2.append(_pack_layer(rows2_all[c], (r - w2id * w2), w2id, ea[es],
                                   nw2, klo2, khi2, co2, 1 << 31))

    # --- weights ---
    W1 = np.asarray(inputs["W1"], np.float32); b1 = np.asarray(inputs["b1"], np.float32)
    W2 = np.asarray(inputs["W2"], np.float32); b2 = np.asarray(inputs["b2"], np.float32)
    Wl1 = np.asarray(inputs["Wl1"], np.float32); bl1 = np.asarray(inputs["bl1"], np.float32)
    Wr1 = np.asarray(inputs["Wr1"], np.float32); br1 = np.asarray(inputs["br1"], np.float32)
    We1 = np.asarray(inputs["We1"], np.float32)
    att1 = np.asarray(inputs["att1"], np.float32)
    bias1 = np.asarray(inputs["bias1"], np.float32)
    Wl2 = np.asarray(inputs["Wl2"], np.float32); bl2 = np.asarray(inputs["bl2"], np.float32)
    Wr2 = np.asarray(inputs["Wr2"], np.float32); br2 = np.asarray(inputs["br2"], np.float32)
    We2 = np.asarray(inputs["We2"], np.float32)
    att2 = np.asarray(inputs["att2"], np.float32)
    bias2 = np.asarray(inputs["bias2"], np.float32)

    consts = {}
    consts["ident"] = _bf(np.eye(P, dtype=np.float32))
    consts["iota"] = _bf(np.tile(np.arange(P, dtype=np.float32), (P, 1)))
    consts["ones1"] = _bf(np.ones((1, P), np.float32))
    consts["mlp1"] = _bf(np.concatenate([W1, b1[None, :]], 0))
    consts["mlp2"] = _bf(np.concatenate([W2, b2[None, :]], 0))

    arow1 = att1.reshape(HC)
    brow1 = (bl1 + br1)[None, :]

    r1c = np.zeros((P, HC), np.float32)
    r1c[0:HID] = Wl1
    r1c[HID:HID + ED] = We1
    r1c[127] = brow1[0]
    consts["rhs1c"] = _bf(r1c)  # full 128 rows; Dsel rows 72:127 overwritten per window
    consts["wr1"] = _bf(Wr1)
    consts["att1t"] = _bf(np.tile(arow1[None, :], (P, 1)))
    rx = np.zeros((P, HC), np.float32)
    rx[0:HID] = Wl1
    rx[127] = bl1 + bias1
    consts["rhsxl1"] = _bf(rx)

    arow2 = att2.reshape(HC)
    brow2 = (br2 - bias2)[None, :]
    r2c = np.zeros((P, HC), np.float32)
    r2c[0:ED] = We2
    r2c[127] = brow2[0]
    consts["rhs2c"] = _bf(r2c)
    consts["wr2a"], consts["wr2b"] = _bf(Wr2[0:P]), _bf(Wr2[P:2 * P])
    consts["wl2a"], consts["wl2b"] = _bf(Wl2[0:P]), _bf(Wl2[P:2 * P])
    xb = (bl2 + bias2)[None, :]
    consts["xl2bias"] = _bf(xb)
    consts["att2b"] = _bf(np.tile(arow2[None, :], (P, 1)))

    nch0 = _ceil_div(n, 512)
    npad = nch0 * 512
    xt_base = np.concatenate([x.T, np.ones((1, n), np.float32)], 0)

    in_maps = []
    for c in range(NCORES):
        lo = c * npc
        rot = np.concatenate([lo + perms[c], (lo + np.arange(npc, n)) % n])
        xt = np.zeros((F_IN + 1, npad), np.float16)
        xt[:, :n] = _bf(xt_base[:, rot])
        m = dict(consts)
        m["xt"] = xt
        m["idx1"], m["dstl1"], m["ea1"] = packed1[c]
        m["idx2"], m["dstl2"], m["ea2"] = packed2[c]
        in_maps.append(m)

    meta = dict(n=n, npc=npc, npad=npad, nch0=nch0,
                w1=w1, nw1=nw1, w2=w2, nw2=nw2,
                klo1=klo1, khi1=khi1, co1=co1,
                klo2=klo2, khi2=khi2, co2=co2,
                agw=agw, agb=agb, b2s=b2s)
    return meta, in_maps, perms


# ----------------------------------------------------------------------------
# device program
# ----------------------------------------------------------------------------

def _build_nc(meta, debug=False):
    n, npc, npad, nch0 = meta["n"], meta["npc"], meta["npad"], meta["nch0"]
    w1, nw1 = meta["w1"], meta["nw1"]
    w2, nw2 = meta["w2"], meta["nw2"]
    klo1, khi1, co1 = meta["klo1"], meta["khi1"], meta["co1"]
    klo2, khi2, co2 = meta["klo2"], meta["khi2"], meta["co2"]
    tc1, tc2 = co1[-1], co2[-1]

    agw, agb = meta["agw"], meta["agb"]

    nc = bacc.Bacc("TRN2", target_bir_lowering=False, num_devices=NCORES)

    def din(name, shape, dtype=BF16):
        return nc.dram_tensor(name, shape, dtype, kind="ExternalInput")

    ident_d = din("ident", [P, P])
    iota_d = din("iota", [P, P])
    ones1_d = din("ones1", [1, P])
    mlp1_d = din("mlp1", [F_IN + 1, HID])
    mlp2_d = din("mlp2", [HID + 1, HID])
    rhs1c_d = din("rhs1c", [P, HC])
    wr1_d = din("wr1", [HID, HC])
    att1t_d = din("att1t", [P, HC])
    rhsxl1_d = din("rhsxl1", [P, HC])
    rhs2c_d = din("rhs2c", [P, HC])
    wr2a_d = din("wr2a", [P, HC]); wr2b_d = din("wr2b", [P, HC])
    wl2a_d = din("wl2a", [P, HC]); wl2b_d = din("wl2b", [P, HC])
    xl2bias_d = din("xl2bias", [1, HC])
    att2b_d = din("att2b", [P, HC])
    xt_d = din("xt", [F_IN + 1, npad])
    idx1_d = din("idx1", [P, tc1], I32)
    dstl1_d = din("dstl1", [P, tc1], F32)
    ea1_d = din("ea1", [P, tc1 * ED])
    idx2_d = din("idx2", [P, tc2], I32)
    dstl2_d = din("dstl2", [P, tc2], F32)
    ea2_d = din("ea2", [P, tc2 * ED])
    out_d = nc.dram_tensor("out", [npc, HC], F32, kind="ExternalOutput")
    if debug:
        dbg_h = nc.dram_tensor("dbg_h", [npad, HID], BF16, kind="ExternalOutput")
        dbg_h1 = nc.dram_tensor("dbg_h1", [npc, HC], BF16, kind="ExternalOutput")
        dbg_xf = nc.dram_tensor("dbg_xf", [NCORES * npc, HC], BF16, kind="ExternalOutput")

    k1max = max(klo1[w] + khi1[w] for w in range(nw1))
    k2max = max(klo2[w] + khi2[w] for w in range(nw2))

    with tile.TileContext(nc) as tc:
        with (
            tc.tile_pool(name="dram", bufs=1, space="DRAM") as dram,
            tc.tile_pool(name="const", bufs=1) as cpool,
            tc.tile_pool(name="win", bufs=3) as wpool,
            tc.tile_pool(name="chunk", bufs=4) as kpool,
            tc.tile_pool(name="ps2", bufs=4, space="PSUM") as ps2,
            tc.tile_pool(name="pst", bufs=2, space="PSUM") as ps_t,
            tc.tile_pool(name="ps1", bufs=1, space="PSUM") as ps1,
        ):
            h_full = dram.tile([npad, HID], BF16)
            h1loc = dram.tile([npc, HC], BF16)
            xl2loc = dram.tile([npc, HC], BF16)
            xl2full = dram.tile([NCORES * npc, HC], BF16)

            def cload(name, shape, dt, src_d):
                t = cpool.tile(shape, dt, tag=name)
                nc.sync.dma_start(t[:], src_d[:, :])
                return t

            ident = cload("ident", [P, P], BF16, ident_d)
            iota = cload("iota", [P, P], BF16, iota_d)
            ones1 = cload("ones1", [1, P], BF16, ones1_d)
            mlp1 = cload("mlp1", [F_IN + 1, HID], BF16, mlp1_d)
            mlp2 = cload("mlp2", [HID + 1, HID], BF16, mlp2_d)
            rhs1c = cload("rhs1c", [P, HC], BF16, rhs1c_d)
            wr1 = cload("wr1", [HID, HC], BF16, wr1_d)
            att1t = cload("att1t", [P, HC], BF16, att1t_d)
            rhsxl1 = cload("rhsxl1", [P, HC], BF16, rhsxl1_d)
            rhs2c = cload("rhs2c", [P, HC], BF16, rhs2c_d)
            wr2a = cload("wr2a", [P, HC], BF16, wr2a_d)
            wr2b = cload("wr2b", [P, HC], BF16, wr2b_d)
            wl2a = cload("wl2a", [P, HC], BF16, wl2a_d)
            wl2b = cload("wl2b", [P, HC], BF16, wl2b_d)
            xl2bias = cload("xl2bias", [1, HC], BF16, xl2bias_d)
            att2b = cload("att2b", [P, HC], BF16, att2b_d)
            zeros = cpool.tile([P, HC + H], BF16)
            nc.vector.memset(zeros[:], 0.0)
            neg4 = cpool.tile([P, 1], F32)
            nc.vector.memset(neg4[:], -4.0)

            # ---------------- phase 0: MLP encoder -> h_full ----------------
            for i in range(nch0):
                sl = slice(i * 512, (i + 1) * 512)
                rx = kpool.tile([F_IN + 1, 512], BF16, tag="mlp_rx")
                nc.sync.dma_start(rx[:], xt_d[:, sl])
                p1 = ps1.tile([HID, 512], F32, tag="ps_prep")
                nc.tensor.matmul(p1[:], lhsT=mlp1[:], rhs=rx[:], start=True, stop=True)
                ht = kpool.tile([HID + 1, 512], BF16, tag="mlp_ht")
                nc.scalar.activation(ht[0:HID, :], p1[:], AF.Relu)
                nc.vector.memset(ht[HID:HID + 1, :], 1.0)
                p2 = ps1.tile([HID, 512], F32, tag="ps_prep")
                nc.tensor.matmul(p2[:], lhsT=mlp2[:], rhs=ht[:], start=True, stop=True)
                h2 = kpool.tile([HID, 512], BF16, tag="mlp_h2")
                nc.scalar.activation(h2[:], p2[:], AF.Relu)
                hrow = kpool.tile([P, 4, HID], BF16, tag="mlp_hrow")
                for j in range(4):
                    pt = ps_t.tile([P, HID], BF16, tag="pst")
                    nc.tensor.transpose(pt[:], h2[:, j * P:(j + 1) * P], ident[0:HID, 0:HID])
                    nc.scalar.activation(hrow[:, j, :], pt[:], AF.Copy)
                nc.sync.dma_start(
                    h_full[sl, :].rearrange("(j p) d -> p j d", p=P), hrow[:])

            # ---------------- phase 1: GAT layer 1 --------------------------
            ab_sl1 = [(h * HID, (h + 1) * HID) for h in range(H)]
            for w in range(nw1):
                span = min(w1, npc - w * w1)
                nb = w * w1
                hw = wpool.tile([w1, HID], BF16, tag="hw")
                nc.sync.dma_start(hw[0:span, :], h_full[nb:nb + span, :])
                pt = ps_t.tile([HID, w1], BF16, tag="pst")
                nc.tensor.transpose(pt[:, 0:span], hw[0:span, :], ident[0:span, 0:span])
                hwT = wpool.tile([HID, w1], BF16, tag="hwT")
                nc.scalar.activation(hwT[:, 0:span], pt[:, 0:span], AF.Copy)
                pxr = ps1.tile([w1, HC], F32, tag="ps_prep")
                nc.tensor.matmul(pxr[0:span, :], lhsT=hwT[:, 0:span], rhs=wr1[:],
                                 start=True, stop=True)
                rstk = wpool.tile([P, HC], BF16, tag="rstk")
                nc.scalar.activation(rstk[:], rhs1c[:], AF.Copy)
                xrw = wpool.tile([w1, HC], BF16, tag="xrw")
                nc.scalar.activation(xrw[0:span, :], pxr[0:span, :], AF.Copy)
                nc.sync.dma_start(rstk[72:72 + span, :], xrw[0:span, :])

                k1 = klo1[w] + khi1[w]
                c0 = co1[w]
                dstw = wpool.tile([P, k1], F32, tag="dstw")
                nc.sync.dma_start(dstw[:], dstl1_d[:, c0:c0 + k1])
                srcw = wpool.tile([P, k1max], I32, tag="srcw")
                nc.sync.dma_start(srcw[:, 0:k1], idx1_d[:, c0:c0 + k1])
                pre = wpool.tile([P, k1max, P], BF16, tag="pre1")
                for c in range(k1):
                    nc.gpsimd.indirect_dma_start(
                        out=pre[:, c, 0:HID], out_offset=None,
                        in_=h_full[:, :],
                        in_offset=IndirectOffsetOnAxis(ap=srcw[:, c:c + 1], axis=0))
                nc.sync.dma_start(
                    pre[:, 0:k1, HID:HID + ED],
                    ea1_d[:, c0 * ED:(c0 + k1) * ED].rearrange(
                        "p (k d) -> p k d", d=ED))
                nc.vector.memset(pre[:, 0:k1, 127:P], 1.0)
                for c in range(k1):
                    nc.vector.tensor_scalar(
                        out=pre[:, c, 72:127], in0=iota[:, 0:55],
                        scalar1=dstw[:, c:c + 1], scalar2=None, op0=ALU.is_equal)

                pout = ps1.tile([P, HC + H], F32, tag="ps_out")
                for c in range(k1):
                    pt2 = ps_t.tile([P, P], BF16, tag="pst")
                    nc.tensor.transpose(pt2[:], pre[:, c, :], ident[:])
                    stk = kpool.tile([P, P], BF16, tag="stk")
                    nc.scalar.activation(stk[:], pt2[:], AF.Copy)
                    psc = ps2.tile([P, 2 * HC], F32, tag="ps_s")
                    nc.tensor.matmul(psc[:, 0:HC], lhsT=stk[:], rhs=rstk[:],
                                     start=True, stop=True)
                    nc.tensor.matmul(psc[:, HC:2 * HC], lhsT=stk[:], rhs=rhsxl1[:],
                                     start=True, stop=True)
                    lr = kpool.tile([P, HC], BF16, tag="abss")
                    nc.scalar.activation(lr[:], psc[:, 0:HC], AF.Prelu, alpha=0.2)
                    scr = kpool.tile([P, HC], BF16, tag="scr")
                    alpha = kpool.tile([P, H], F32, tag="alpha")
                    for h in range(H):
                        a, b = h * HID, (h + 1) * HID
                        nc.vector.scalar_tensor_tensor(
                            out=scr[:, a:b], in0=lr[:, a:b], scalar=1.0,
                            in1=att1t[:, a:b], op0=ALU.mult, op1=ALU.mult,
                            accum_out=alpha[:, h:h + 1])
                    ex = kpool.tile([P, H], F32, tag="ex")
                    nc.scalar.activation(ex[:], alpha[:], AF.Exp, bias=neg4[:])
                    pay = kpool.tile([P, HC + H], BF16, tag="pay")
                    for h, (a, b) in enumerate(ab_sl1):
                        if h < 2:
                            nc.scalar.activation(pay[:, a:b], psc[:, HC + a:HC + b],
                                                 AF.Copy, scale=ex[:, h:h + 1])
                        else:
                            nc.vector.tensor_scalar(
                                out=pay[:, a:b], in0=psc[:, HC + a:HC + b],
                                scalar1=ex[:, h:h + 1], scalar2=None, op0=ALU.mult)
                    nc.vector.tensor_copy(pay[:, HC:HC + H], ex[:])
                    nc.tensor.matmul(pout[0:span, :], lhsT=pre[:, c, 72:72 + span],
                                     rhs=pay[:], start=(c == 0), stop=(c == k1 - 1))

                deng = wpool.tile([w1, H], F32, tag="deng")
                nc.vector.tensor_scalar(out=deng[0:span, :], in0=pout[0:span, HC:HC + H],
                                        scalar1=1e-30, scalar2=None, op0=ALU.max)
                rden = wpool.tile([w1, H], F32, tag="rden")
                nc.vector.reciprocal(rden[0:span, :], deng[0:span, :])
                h1w = wpool.tile([w1, HC], BF16, tag="h1w")
                for h, (a, b) in enumerate(ab_sl1):
                    nc.vector.tensor_scalar(
                        out=h1w[0:span, a:b], in0=pout[0:span, a:b],
                        scalar1=rden[0:span, h:h + 1], scalar2=0.0,
                        op0=ALU.mult, op1=ALU.max)
                nc.sync.dma_start(h1loc[nb:nb + span, :], h1w[0:span, :])

                pxt = ps_t.tile([P, w1], BF16, tag="pst")
                h1T0 = wpool.tile([P, w1], BF16, tag="h1T0")
                nc.tensor.transpose(pxt[:, 0:span], h1w[0:span, 0:P], ident[0:span, 0:span])
                nc.scalar.activation(h1T0[:, 0:span], pxt[:, 0:span], AF.Copy)
                pxt2 = ps_t.tile([P, w1], BF16, tag="pst")
                h1T1 = wpool.tile([P, w1], BF16, tag="h1T1")
                nc.tensor.transpose(pxt2[:, 0:span], h1w[0:span, P:HC], ident[0:span, 0:span])
                nc.scalar.activation(h1T1[:, 0:span], pxt2[:, 0:span], AF.Copy)
                pxl2 = ps1.tile([w1, HC], F32, tag="ps_prep")
                nc.tensor.matmul(pxl2[0:span, :], lhsT=h1T0[:, 0:span], rhs=wl2a[:],
                                 start=True, stop=False)
                nc.tensor.matmul(pxl2[0:span, :], lhsT=h1T1[:, 0:span], rhs=wl2b[:],
                                 start=False, stop=False)
                nc.tensor.matmul(pxl2[0:span, :], lhsT=ones1[:, 0:span], rhs=xl2bias[:],
                                 start=False, stop=True)
                xl2w = wpool.tile([w1, HC], BF16, tag="xl2w")
                nc.scalar.activation(xl2w[0:span, :], pxl2[0:span, :], AF.Copy)
                nc.sync.dma_start(xl2loc[nb:nb + span, :], xl2w[0:span, :])

                # fire this AllGather chunk as soon as its windows are written
                if (w + 1) in agw:
                    j = agw.index(w + 1) - 1
                    lo_r, hi_r = agb[j], agb[j + 1]
                    nc.gpsimd.collective_compute(
                        "AllGather", ALU.bypass,
                        replica_groups=[list(range(NCORES))],
                        ins=[xl2loc[lo_r:hi_r]],
                        outs=[xl2full[NCORES * lo_r:NCORES * hi_r]])

            # ---------------- phase 3: GAT layer 2 --------------------------
            for w in range(nw2):
                span = min(w2, npc - w * w2)
                nb = w * w2
                h1r = wpool.tile([w2, HC], BF16, tag="h1r")
                nc.sync.dma_start(h1r[0:span, :], h1loc[nb:nb + span, :])
                pt0 = ps_t.tile([P, w2], BF16, tag="pst")
                hrT0 = wpool.tile([P, w2], BF16, tag="hrT0")
                nc.tensor.transpose(pt0[:, 0:span], h1r[0:span, 0:P], ident[0:span, 0:span])
                nc.scalar.activation(hrT0[:, 0:span], pt0[:, 0:span], AF.Copy)
                pt1 = ps_t.tile([P, w2], BF16, tag="pst")
                hrT1 = wpool.tile([P, w2], BF16, tag="hrT1")
                nc.tensor.transpose(pt1[:, 0:span], h1r[0:span, P:HC], ident[0:span, 0:span])
                nc.scalar.activation(hrT1[:, 0:span], pt1[:, 0:span], AF.Copy)
                pxr2 = ps1.tile([w2, HC], F32, tag="ps_prep")
                nc.tensor.matmul(pxr2[0:span, :], lhsT=hrT0[:, 0:span], rhs=wr2a[:],
                                 start=True, stop=False)
                nc.tensor.matmul(pxr2[0:span, :], lhsT=hrT1[:, 0:span], rhs=wr2b[:],
                                 start=False, stop=True)
                rstk2 = wpool.tile([P, HC], BF16, tag="rstk2")
                nc.scalar.activation(rstk2[:], rhs2c[:], AF.Copy)
                xrw2 = wpool.tile([w2, HC], BF16, tag="xrw2")
                nc.scalar.activation(xrw2[0:span, :], pxr2[0:span, :], AF.Copy)
                nc.sync.dma_start(rstk2[ED:ED + span, :], xrw2[0:span, :])

                k2 = klo2[w] + khi2[w]
                c0 = co2[w]
                dstw2 = wpool.tile([P, k2], F32, tag="dstw2")
                nc.sync.dma_start(dstw2[:], dstl2_d[:, c0:c0 + k2])
                pre2 = wpool.tile([P, k2max, P], BF16, tag="pre2")
                nc.sync.dma_start(
                    pre2[:, 0:k2, 0:ED],
                    ea2_d[:, c0 * ED:(c0 + k2) * ED].rearrange(
                        "p (k d) -> p k d", d=ED))
                nc.vector.memset(pre2[:, 0:k2, 127:P], 1.0)
                for c in range(k2):
                    nc.vector.tensor_scalar(
                        out=pre2[:, c, ED:ED + 119], in0=iota[:, 0:119],
                        scalar1=dstw2[:, c:c + 1], scalar2=None, op0=ALU.is_equal)
                srcw2 = wpool.tile([P, k2max], I32, tag="srcw2")
                nc.sync.dma_start(srcw2[:, 0:k2], idx2_d[:, c0:c0 + k2])
                xg = wpool.tile([P, k2max, HC], BF16, tag="xg")
                for c in range(k2):
                    nc.gpsimd.indirect_dma_start(
                        out=xg[:, c, :], out_offset=None,
                        in_=xl2full[:, :],
                        in_offset=IndirectOffsetOnAxis(ap=srcw2[:, c:c + 1], axis=0))

                pout2 = ps1.tile([P, HC + 1], F32, tag="ps_out")
                for c in range(k2):
                    pt2 = ps_t.tile([P, P], BF16, tag="pst")
                    nc.tensor.transpose(pt2[:], pre2[:, c, :], ident[:])
                    stk2 = kpool.tile([P, P], BF16, tag="stk")
                    nc.vector.tensor_copy(stk2[:], pt2[:])
                    psc2 = ps2.tile([P, 2 * HC], F32, tag="ps_s")
                    nc.tensor.matmul(psc2[:, 0:HC], lhsT=stk2[:], rhs=rstk2[:],
                                     start=True, stop=False)
                    nc.tensor.matmul(psc2[:, 0:P], lhsT=ident[:], rhs=xg[:, c, 0:P],
                                     start=False, stop=False)
                    nc.tensor.matmul(psc2[:, P:HC], lhsT=ident[:], rhs=xg[:, c, P:HC],
                                     start=False, stop=True)
                    lr2 = kpool.tile([P, HC], BF16, tag="abss")
                    nc.scalar.activation(lr2[:], psc2[:, 0:HC], AF.Prelu, alpha=0.2)
                    scr2 = kpool.tile([P, HC], BF16, tag="scr")
                    alpha2 = kpool.tile([P, 1], F32, tag="alpha")
                    nc.vector.scalar_tensor_tensor(
                        out=scr2[:], in0=lr2[:], scalar=1.0,
                        in1=att2b[:], op0=ALU.mult, op1=ALU.mult,
                        accum_out=alpha2[:])
                    ex2 = kpool.tile([P, 1], F32, tag="ex")
                    nc.scalar.activation(ex2[:], alpha2[:], AF.Exp,
                                         bias=neg4[:], scale=1.0)
                    pay2 = kpool.tile([P, HC + 1], BF16, tag="pay")
                    nc.scalar.activation(pay2[:, 0:P], xg[:, c, 0:P], AF.Copy,
                                         scale=ex2[:])
                    nc.vector.tensor_scalar(
                        out=pay2[:, P:HC], in0=xg[:, c, P:HC],
                        scalar1=ex2[:], scalar2=None, op0=ALU.mult)
                    nc.vector.tensor_copy(pay2[:, HC:HC + 1], ex2[:])
                    nc.tensor.matmul(pout2[0:span, :], lhsT=pre2[:, c, ED:ED + span],
                                     rhs=pay2[:], start=(c == 0), stop=(c == k2 - 1))

                deng2 = wpool.tile([w2, 1], F32, tag="deng")
                nc.vector.tensor_scalar(out=deng2[0:span, :], in0=pout2[0:span, HC:HC + 1],
                                        scalar1=1e-30, scalar2=None, op0=ALU.max)
                rden2 = wpool.tile([w2, 1], F32, tag="rden")
                nc.vector.reciprocal(rden2[0:span, :], deng2[0:span, :])
                outw = wpool.tile([w2, HC], F32, tag="outw")
                nc.vector.tensor_scalar(
                    out=outw[0:span, :], in0=pout2[0:span, 0:HC],
                    scalar1=rden2[0:span, :], scalar2=0.0, op0=ALU.mult, op1=ALU.max)
                nc.sync.dma_start(out_d[nb:nb + span, :], outw[0:span, :])

            if debug:
                nc.sync.dma_start(dbg_h[:, :], h_full[:, :])
                nc.sync.dma_start(dbg_h1[:, :], h1loc[:, :])
                nc.sync.dma_start(dbg_xf[:, :], xl2full[:, :])

    nc.finalize()
    return nc


# ----------------------------------------------------------------------------
# entry point
# ----------------------------------------------------------------------------

def _install_ntff_hook():
    """Shim antenv.axon_hooks so trace=True can collect NTFF profiles."""
    import types
    try:
        from antenv.axon_hooks import get_axon_ntff_profile_hook  # noqa: F401
        return
    except ImportError:
        pass
    try:
        import antenv
        boot_dir = "/root/.axon_site/trn_agent_boot"
        so_path = "/opt/axon/libaxon_pjrt.so"
        if boot_dir not in sys.path:
            sys.path.insert(0, boot_dir)
        import trn_boot
        mod = types.ModuleType("antenv.axon_hooks")
        _state = {"hook": None}
        mod.set_axon_ntff_profile_hook = lambda h: _state.__setitem__("hook", h)
        mod.get_axon_ntff_profile_hook = lambda: _state["hook"]
        sys.modules["antenv.axon_hooks"] = mod
        antenv.axon_hooks = mod
        if os.path.exists(so_path):
            mod.set_axon_ntff_profile_hook(
                trn_boot._ntff_profile_via_ctypes(so_path))
    except Exception as exc:  # profiling is best-effort
        print("ntff hook install failed:", exc)


def run(inputs, trace=False):
    if trace:
        _install_ntff_hook()
    n = int(inputs["x"].shape[0])
    e = int(inputs["edge_index"].shape[1])
    assert n % NCORES == 0
    npc = n // NCORES
    meta, in_maps, perms = _prep_host(inputs, n, e, npc, w1=55, w2=119)
    nc = _build_nc(meta)
    res = run_bass_kernel_spmd(nc, in_maps, list(range(NCORES)), trace=trace)
    full = np.empty((n, HC), np.float32)
    for c in range(NCORES):
        full[c * npc + perms[c]] = res.results[c]["out"]
    return full, res


def kernel(**inputs):
    full, _ = run(inputs, trace=False)
    return full

